# revision 19
# baseline (speedup 1.0000x reference)
"""Per-env MLP (EnvironVectorField) Trainium2 kernel.

Reference computation (fp32):
    x = u.reshape(B, E, D)  # B=16384, E=8 envs, D=64
    h = swish(x @ W1[e] + b1[e]); h = swish(h @ W2[e] + b2[e])
    h = swish(h @ W3[e] + b3[e]); out = h @ W4[e] + b4[e]
    return out.reshape(B*E, D)

Sharding: expert-parallel — core e computes env e entirely (u rows e::8).

Per-core design: activations are kept feature-major (features on SBUF
partitions, batch on the free axis) so weights are the stationary matmul
operand and every weight tile is reused across the whole batch. Input and
output tiles are transposed on the tensor engine via identity matmuls.
Matmuls run in float32r (rounded fp32, ~1e-4 rel err), which streams at
1 cycle/row; full fp32 would be 4x slower.

The batch is processed in chunks of NB columns. Layer 1 of chunk c+1 is
interleaved into layer 3 of chunk c (three rotating h buffers) because
layer 1 has only 2 matmuls per PSUM evacuation and would otherwise stall
the tensor engine behind the scalar engine's Silu evacuations.
"""

import sys

sys.path.insert(0, '/opt/trn_rl_repo')

from contextlib import ExitStack

import numpy as np

import concourse.bacc as bacc
import concourse.bass as bass
import concourse.mybir as mybir
import concourse.tile as tile
from concourse.bass_utils import run_bass_kernel_spmd
from concourse.masks import make_identity

F32 = mybir.dt.float32
F32R = mybir.dt.float32r
SILU = mybir.ActivationFunctionType.Silu
IDENT = mybir.ActivationFunctionType.Identity

N_ENV = 8
D = 64          # state dim
H = 1024        # hidden dim
B = 16384       # rows per env
NB = 1024       # batch-chunk columns per chunk
NCH = B // NB   # 16 chunks
NT = NB // 512  # 512-wide matmul n-tiles per chunk
KT = H // 128   # 8 k/m tiles of 128 over the hidden dim


def build_module(iters: int = 1, rowpack: bool = False):
    nc = bacc.Bacc("TRN2", target_bir_lowering=False, num_devices=N_ENV)

    xin = nc.dram_tensor("x", (B, D), F32, kind="ExternalInput")
    w1 = nc.dram_tensor("w1", (128, H), F32, kind="ExternalInput")       # [Kpad, M]
    w2 = nc.dram_tensor("w2", (128, KT, H), F32, kind="ExternalInput")   # [ki, ko, M]
    w3 = nc.dram_tensor("w3", (128, KT, H), F32, kind="ExternalInput")
    w4 = nc.dram_tensor("w4", (128, KT, D), F32, kind="ExternalInput")
    b1 = nc.dram_tensor("b1", (128, KT), F32, kind="ExternalInput")      # [mi, mo]
    b2 = nc.dram_tensor("b2", (128, KT), F32, kind="ExternalInput")
    b3 = nc.dram_tensor("b3", (128, KT), F32, kind="ExternalInput")
    b4 = nc.dram_tensor("b4", (D, 1), F32, kind="ExternalInput")
    yout = nc.dram_tensor("y", (B, D), F32, kind="ExternalOutput")

    xv = xin.rearrange("(c j p) d -> c p j d", p=128, j=KT)
    yv = yout.rearrange("(c j p) d -> c p j d", p=128, j=KT)

    with tile.TileContext(nc) as tc, ExitStack() as ctx:
        wpool = ctx.enter_context(tc.tile_pool(name="wpool", bufs=1))
        wstage = ctx.enter_context(tc.tile_pool(name="wstage", bufs=2))
        mps = ctx.enter_context(tc.tile_pool(name="mps", bufs=2, space="PSUM"))
        lps = ctx.enter_context(tc.tile_pool(name="lps", bufs=1, space="PSUM"))
        tpp = ctx.enter_context(tc.tile_pool(name="tpp", bufs=2, space="PSUM"))

        ident = wpool.tile([128, 128], F32)
        make_identity(nc, ident)

        # xT zero-fill first: rows D..127 stay zero (K=64 padded to 128)
        xT0 = wpool.tile([128, NB], F32R)
        xT1 = wpool.tile([128, NB], F32R)
        zsrc = wstage.tile([128, 2, H], F32, tag="wtmp", name="zsrc")
        nc.any.memzero(zsrc[:])
        nc.vector.tensor_copy(xT0[:], zsrc[:, 0, :])
        nc.vector.tensor_copy(xT1[:], zsrc[:, 0, :])

        # biases in one padded tile
        ball = wpool.tile([128, 3 * KT + 1], F32)
        nc.sync.dma_start(ball[:, 0:KT], b1[:])
        nc.sync.dma_start(ball[:, KT:2 * KT], b2[:])
        nc.sync.dma_start(ball[:, 2 * KT:3 * KT], b3[:])
        nc.sync.dma_start(ball[:D, 3 * KT:3 * KT + 1], b4[:])
        b1s = ball[:, 0:KT]
        b2s = ball[:, KT:2 * KT]
        b3s = ball[:, 2 * KT:3 * KT]
        b4s = ball[:D, 3 * KT:3 * KT + 1]

        # weights: DMA f32 pieces through an 8KB staging slot, cast to f32r
        w1r = wpool.tile([128, H], F32R)
        w2r = wpool.tile([128, KT, H], F32R)
        w3r = wpool.tile([128, KT, H], F32R)
        w4r = wpool.tile([128, KT, D], F32R)

        t1 = wstage.tile([128, 1, H], F32, tag="wtmp", name="wt_w1")
        nc.sync.dma_start(t1[:, 0, :], w1[:])
        nc.vector.tensor_copy(w1r[:], t1[:, 0, :])
        t4 = wstage.tile([128, KT, D], F32, tag="wtmp", name="wt_w4")
        nc.sync.dma_start(t4[:], w4[:])
        nc.vector.tensor_copy(w4r[:], t4[:])

        def load_pieces(dram3, dst, nm):
            for p in range(KT // 2):
                tmp = wstage.tile([128, 2, H], F32, tag="wtmp", name=f"wt_{nm}_{p}")
                nc.sync.dma_start(tmp[:], dram3[:, 2 * p:2 * p + 2, :])
                nc.vector.tensor_copy(dst[:, 2 * p:2 * p + 2, :], tmp[:])

        load_pieces(w2, w2r, "w2")
        load_pieces(w3, w3r, "w3")

        # persistent activation buffers (fixed roles)
        hA = wpool.tile([128, KT, NB], F32R)   # L1 out
        hB = wpool.tile([128, KT, NB], F32R)   # L2 out
        hC = wpool.tile([128, KT, NB], F32R)   # L3 out
        oT = wpool.tile([D, NB], F32)          # L4 out
        xball = wpool.tile([128, 2, KT, D], F32)
        oball = wpool.tile([128, 2, KT, D], F32)

        def dma_in(c, it=0):
            nc.sync.dma_start(xball[:, c % 2], xv[c])

        def transposes_in(c, it=0):
            xT = (xT0, xT1)[c % 2]
            for j in range(KT):
                tp = tpp.tile([D, 128], F32, tag="tp", name=f"tp_{it}_{c}_{j}")
                nc.tensor.transpose(tp[:], xball[:, c % 2, j, :], ident[:])
                nc.vector.tensor_copy(xT[:D, j * 128:(j + 1) * 128], tp[:])
                if rowpack:
                    nc.vector.tensor_copy(xT[D:128, j * 128:(j + 1) * 128], tp[:])

        def l1_pair(c, mp, it=0):
            if not rowpack:
                xT = (xT0, xT1)[c % 2]
                for m in (2 * mp, 2 * mp + 1):
                    for n in range(NT):
                        pm = mps.tile([128, 512], F32, tag=f"mm{n}",
                                      name=f"p1_{it}_{c}_{m}_{n}")
                        nc.tensor.matmul(pm[:], w1r[:, m * 128:(m + 1) * 128],
                                         xT[:, n * 512:(n + 1) * 512],
                                         start=True, stop=True)
                        nc.scalar.activation(hA[:, m, n * 512:(n + 1) * 512], pm[:],
                                             SILU, bias=b1s[:, m:m + 1])
                return
            # two m-tiles computed concurrently: K=64 row groups 0 and 64
            xT = (xT0, xT1)[c % 2]
            mA, mB = 2 * mp, 2 * mp + 1
            for n in range(NT):
                pa = mps.tile([128, 512], F32, tag=f"mm{n}", name=f"p1a_{it}_{c}_{mp}_{n}")
                pb = mps.tile([128, 512], F32, tag=f"mm{n}", name=f"p1b_{it}_{c}_{mp}_{n}")
                nc.tensor.matmul(pa[:], w1r[:D, mA * 128:(mA + 1) * 128],
                                 xT[:D, n * 512:(n + 1) * 512], start=True, stop=True,
                                 tile_position=(0, 0))
                nc.tensor.matmul(pb[:], w1r[D:128, mB * 128:(mB + 1) * 128],
                                 xT[D:128, n * 512:(n + 1) * 512], start=True, stop=True,
                                 tile_position=(64, 0))
                nc.scalar.activation(hA[:, mA, n * 512:(n + 1) * 512], pa[:],
                                     SILU, bias=b1s[:, mA:mA + 1])
                nc.scalar.activation(hA[:, mB, n * 512:(n + 1) * 512], pb[:],
                                     SILU, bias=b1s[:, mB:mB + 1])

        def mid_group(li, wr, bs, hs, hd, c, m, it=0):
            pms = [mps.tile([128, 512], F32, tag=f"mm{n}",
                            name=f"p{li}_{it}_{c}_{m}_{n}") for n in range(NT)]
            for k in range(KT):
                for n in range(NT):
                    nc.tensor.matmul(pms[n][:], wr[:, k, m * 128:(m + 1) * 128],
                                     hs[:, k, n * 512:(n + 1) * 512],
                                     start=(k == 0), stop=(k == KT - 1))
            for n in range(NT):
                nc.scalar.activation(hd[:, m, n * 512:(n + 1) * 512], pms[n][:],
                                     SILU, bias=bs[:, m:m + 1])

        def tail(c, it=0):
            # L4: hC -> oT, then transpose to batch-major and store
            for n in range(NT):
                ns_ = slice(n * 512, (n + 1) * 512)
                p4 = lps.tile([D, 512], F32, tag=f"l4{n % 2}", name=f"p4_{it}_{c}_{n}")
                for k in range(KT):
                    nc.tensor.matmul(p4[:], w4r[:, k, :], hC[:, k, ns_],
                                     start=(k == 0), stop=(k == KT - 1))
                nc.vector.tensor_scalar_add(oT[:, ns_], p4[:], b4s)
            for j in range(KT):
                tq = tpp.tile([128, D], F32, tag="tp", name=f"tq_{it}_{c}_{j}")
                nc.tensor.transpose(tq[:], oT[:, j * 128:(j + 1) * 128], ident[:D, :D])
                nc.vector.tensor_copy(oball[:, c % 2, j, :], tq[:])
            nc.sync.dma_start(yv[c], oball[:, c % 2])

        def full_pass(it=0):
            dma_in(0, it)
            transposes_in(0, it)
            for mp in range(KT // 2):
                l1_pair(0, mp, it)
            for c in range(NCH):
                if c + 1 < NCH:
                    dma_in(c + 1, it)
                for m in range(KT):
                    mid_group(2, w2r, b2s, hA, hB, c, m, it)
                for m in range(KT):
                    mid_group(3, w3r, b3s, hB, hC, c, m, it)
                    if c + 1 < NCH:
                        if m == 0:
                            transposes_in(c + 1, it)
                        if m % 2 == 1:
                            l1_pair(c + 1, m // 2, it)
                tail(c, it)

        if iters == 1:
            full_pass()
        else:
            with tc.For_i(0, iters, 1):
                full_pass()

    nc.compile()
    return nc


def _prep_in_maps(t, u, W1, b1, W2, b2, W3, b3, W4, b4):
    in_maps = []
    for e in range(N_ENV):
        w1p = np.empty((128, H), np.float32)
        w1p[:D] = W1[e]
        w1p[D:] = W1[e]
        in_maps.append({
            "x": np.ascontiguousarray(u[e::N_ENV]),
            "w1": w1p,
            "w2": np.ascontiguousarray(W2[e].reshape(KT, 128, H).transpose(1, 0, 2)),
            "w3": np.ascontiguousarray(W3[e].reshape(KT, 128, H).transpose(1, 0, 2)),
            "w4": np.ascontiguousarray(W4[e].reshape(KT, 128, D).transpose(1, 0, 2)),
            "b1": np.ascontiguousarray(b1[e].reshape(KT, 128).T),
            "b2": np.ascontiguousarray(b2[e].reshape(KT, 128).T),
            "b3": np.ascontiguousarray(b3[e].reshape(KT, 128).T),
            "b4": np.ascontiguousarray(b4[e].reshape(D, 1)),
        })
    return in_maps


_CACHED_NC = None


def kernel(t, u, W1, b1, W2, b2, W3, b3, W4, b4):
    global _CACHED_NC
    u = np.asarray(u, np.float32)
    args = [np.asarray(a, np.float32) for a in (W1, b1, W2, b2, W3, b3, W4, b4)]
    if _CACHED_NC is None:
        _CACHED_NC = build_module()
    in_maps = _prep_in_maps(None, u, *args)
    res = run_bass_kernel_spmd(_CACHED_NC, in_maps, core_ids=list(range(N_ENV)))
    out = np.empty((B * N_ENV, D), np.float32)
    for e in range(N_ENV):
        out[e::N_ENV] = res.results[e]["y"]
    return out


# revision 21
# speedup vs baseline: 1.1041x; 1.1041x over previous
"""Per-env MLP (EnvironVectorField) Trainium2 kernel.

Reference computation (fp32):
    x = u.reshape(B, E, D)  # B=16384, E=8 envs, D=64
    h = swish(x @ W1[e] + b1[e]); h = swish(h @ W2[e] + b2[e])
    h = swish(h @ W3[e] + b3[e]); out = h @ W4[e] + b4[e]
    return out.reshape(B*E, D)

Sharding: expert-parallel — core e computes env e entirely (u rows e::8).

Per-core design: activations are kept feature-major (features on SBUF
partitions, batch on the free axis) so weights are the stationary matmul
operand and every weight tile is reused across the whole batch. Input and
output tiles are transposed on the tensor engine via identity matmuls.
Matmuls run in float32r (rounded fp32, ~1e-4 rel err), which streams at
1 cycle/row; full fp32 would be 4x slower.

The batch is processed in chunks of NB columns. Layer 1 of chunk c+1 is
interleaved into layer 3 of chunk c (three rotating h buffers) because
layer 1 has only 2 matmuls per PSUM evacuation and would otherwise stall
the tensor engine behind the scalar engine's Silu evacuations.
"""

import sys

sys.path.insert(0, '/opt/trn_rl_repo')

from contextlib import ExitStack

import numpy as np

import concourse.bacc as bacc
import concourse.bass as bass
import concourse.mybir as mybir
import concourse.tile as tile
from concourse.bass_utils import run_bass_kernel_spmd
from concourse.masks import make_identity

F32 = mybir.dt.float32
F32R = mybir.dt.float32r
SILU = mybir.ActivationFunctionType.Silu
IDENT = mybir.ActivationFunctionType.Identity

N_ENV = 8
D = 64          # state dim
H = 1024        # hidden dim
B = 16384       # rows per env
NB = 1024       # batch-chunk columns per chunk
NCH = B // NB   # 16 chunks
NT = NB // 512  # 512-wide matmul n-tiles per chunk
KT = H // 128   # 8 k/m tiles of 128 over the hidden dim


def build_module(iters: int = 1, rowpack: bool = False, fuse: bool = False):
    nc = bacc.Bacc("TRN2", target_bir_lowering=False, num_devices=N_ENV)

    xin = nc.dram_tensor("x", (B, D), F32, kind="ExternalInput")
    w1 = nc.dram_tensor("w1", (128, H), F32, kind="ExternalInput")       # [Kpad, M]
    w2 = nc.dram_tensor("w2", (128, KT, H), F32, kind="ExternalInput")   # [ki, ko, M]
    w3 = nc.dram_tensor("w3", (128, KT, H), F32, kind="ExternalInput")
    w4 = nc.dram_tensor("w4", (128, KT, D), F32, kind="ExternalInput")
    b1 = nc.dram_tensor("b1", (128, KT), F32, kind="ExternalInput")      # [mi, mo]
    b2 = nc.dram_tensor("b2", (128, KT), F32, kind="ExternalInput")
    b3 = nc.dram_tensor("b3", (128, KT), F32, kind="ExternalInput")
    b4 = nc.dram_tensor("b4", (D, 1), F32, kind="ExternalInput")
    yout = nc.dram_tensor("y", (B, D), F32, kind="ExternalOutput")

    xv = xin.rearrange("(c j p) d -> c p j d", p=128, j=KT)
    yv = yout.rearrange("(c j p) d -> c p j d", p=128, j=KT)

    with tile.TileContext(nc) as tc, ExitStack() as ctx:
        wpool = ctx.enter_context(tc.tile_pool(name="wpool", bufs=1))
        wstage = ctx.enter_context(tc.tile_pool(name="wstage", bufs=2))
        mps = ctx.enter_context(tc.tile_pool(name="mps", bufs=3, space="PSUM"))
        tpp = ctx.enter_context(tc.tile_pool(name="tpp", bufs=2, space="PSUM"))

        ident = wpool.tile([128, 128], F32)
        make_identity(nc, ident)

        xball = wpool.tile([128, 2, KT, D], F32)
        if iters == 1:
            # queue chunk-0 input ahead of the 9MB weight DMAs (FIFO queues)
            nc.sync.dma_start(xball[:, 0], xv[0])

        # xT zero-fill first: rows D..127 stay zero (K=64 padded to 128)
        xT0 = wpool.tile([128, NB], F32R)
        xT1 = wpool.tile([128, NB], F32R)
        zsrc = wstage.tile([128, 2, H], F32, tag="wtmp", name="zsrc")
        nc.any.memzero(zsrc[:])
        nc.vector.tensor_copy(xT0[:], zsrc[:, 0, :])
        nc.vector.tensor_copy(xT1[:], zsrc[:, 0, :])

        # biases in one padded tile
        ball = wpool.tile([128, 3 * KT + 1], F32)
        nc.sync.dma_start(ball[:, 0:KT], b1[:])
        nc.sync.dma_start(ball[:, KT:2 * KT], b2[:])
        nc.sync.dma_start(ball[:, 2 * KT:3 * KT], b3[:])
        nc.sync.dma_start(ball[:D, 3 * KT:3 * KT + 1], b4[:])
        b1s = ball[:, 0:KT]
        b2s = ball[:, KT:2 * KT]
        b3s = ball[:, 2 * KT:3 * KT]
        b4s = ball[:D, 3 * KT:3 * KT + 1]

        # weights: DMA f32 pieces through an 8KB staging slot, cast to f32r
        w1r = wpool.tile([128, H], F32R)
        w2r = wpool.tile([128, KT, H], F32R)
        w3r = wpool.tile([128, KT, H], F32R)
        w4r = wpool.tile([128, KT, D], F32R)

        t1 = wstage.tile([128, 1, H], F32, tag="wtmp", name="wt_w1")
        nc.sync.dma_start(t1[:, 0, :], w1[:])
        nc.vector.tensor_copy(w1r[:], t1[:, 0, :])
        t4 = wstage.tile([128, KT, D], F32, tag="wtmp", name="wt_w4")
        nc.sync.dma_start(t4[:], w4[:])
        nc.vector.tensor_copy(w4r[:], t4[:])

        def load_pieces(dram3, dst, nm):
            for p in range(KT // 2):
                tmp = wstage.tile([128, 2, H], F32, tag="wtmp", name=f"wt_{nm}_{p}")
                nc.sync.dma_start(tmp[:], dram3[:, 2 * p:2 * p + 2, :])
                nc.vector.tensor_copy(dst[:, 2 * p:2 * p + 2, :], tmp[:])

        load_pieces(w2, w2r, "w2")
        load_pieces(w3, w3r, "w3")

        # persistent activation buffers (fixed roles)
        hA = wpool.tile([128, KT, NB], F32R)   # L1 out
        hB = wpool.tile([128, KT, NB], F32R)   # L2 out
        hC = wpool.tile([128, KT, NB], F32R)   # L3 out
        oT = wpool.tile([D, NB], F32)          # L4 out
        oball = wpool.tile([128, 2, KT, D], F32)

        def dma_in(c, it=0):
            nc.sync.dma_start(xball[:, c % 2], xv[c])

        def transposes_in(c, it=0):
            xT = (xT0, xT1)[c % 2]
            for j in range(KT):
                tp = tpp.tile([D, 128], F32, tag="tp", name=f"tp_{it}_{c}_{j}")
                nc.tensor.transpose(tp[:], xball[:, c % 2, j, :], ident[:])
                nc.vector.tensor_copy(xT[:D, j * 128:(j + 1) * 128], tp[:])
                if rowpack:
                    nc.vector.tensor_copy(xT[D:128, j * 128:(j + 1) * 128], tp[:])

        def l1_pair(c, mp, it=0):
            if not rowpack:
                xT = (xT0, xT1)[c % 2]
                for m in (2 * mp, 2 * mp + 1):
                    if fuse:
                        pw = mps.tile([128, NB], F32, tag="mmw",
                                      name=f"p1_{it}_{c}_{m}")[:]
                        for n in range(NT):
                            nc.tensor.matmul(pw[:, n * 512:(n + 1) * 512],
                                             w1r[:, m * 128:(m + 1) * 128],
                                             xT[:, n * 512:(n + 1) * 512],
                                             start=True, stop=True)
                        nc.scalar.activation(hA[:, m, :], pw,
                                             SILU, bias=b1s[:, m:m + 1])
                        continue
                    for n in range(NT):
                        pm = mps.tile([128, 512], F32, tag=f"mm{n}",
                                      name=f"p1_{it}_{c}_{m}_{n}")
                        nc.tensor.matmul(pm[:], w1r[:, m * 128:(m + 1) * 128],
                                         xT[:, n * 512:(n + 1) * 512],
                                         start=True, stop=True)
                        nc.scalar.activation(hA[:, m, n * 512:(n + 1) * 512], pm[:],
                                             SILU, bias=b1s[:, m:m + 1])
                return
            # two m-tiles computed concurrently: K=64 row groups 0 and 64
            xT = (xT0, xT1)[c % 2]
            mA, mB = 2 * mp, 2 * mp + 1
            for n in range(NT):
                pa = mps.tile([128, 512], F32, tag=f"mm{n}", name=f"p1a_{it}_{c}_{mp}_{n}")
                pb = mps.tile([128, 512], F32, tag=f"mm{n}", name=f"p1b_{it}_{c}_{mp}_{n}")
                nc.tensor.matmul(pa[:], w1r[:D, mA * 128:(mA + 1) * 128],
                                 xT[:D, n * 512:(n + 1) * 512], start=True, stop=True,
                                 tile_position=(0, 0))
                nc.tensor.matmul(pb[:], w1r[D:128, mB * 128:(mB + 1) * 128],
                                 xT[D:128, n * 512:(n + 1) * 512], start=True, stop=True,
                                 tile_position=(64, 0))
                nc.scalar.activation(hA[:, mA, n * 512:(n + 1) * 512], pa[:],
                                     SILU, bias=b1s[:, mA:mA + 1])
                nc.scalar.activation(hA[:, mB, n * 512:(n + 1) * 512], pb[:],
                                     SILU, bias=b1s[:, mB:mB + 1])

        def mid_group(li, wr, bs, hs, hd, c, m, it=0):
            if fuse:
                pw = mps.tile([128, NB], F32, tag="mmw", name=f"p{li}_{it}_{c}_{m}")[:]
                pms = [pw[:, n * 512:(n + 1) * 512] for n in range(NT)]
            else:
                pms = [mps.tile([128, 512], F32, tag=f"mm{n}",
                                name=f"p{li}_{it}_{c}_{m}_{n}")[:] for n in range(NT)]
            for k in range(KT):
                for n in range(NT):
                    nc.tensor.matmul(pms[n], wr[:, k, m * 128:(m + 1) * 128],
                                     hs[:, k, n * 512:(n + 1) * 512],
                                     start=(k == 0), stop=(k == KT - 1))
            if fuse:
                nc.scalar.activation(hd[:, m, :], pw, SILU, bias=bs[:, m:m + 1])
            else:
                for n in range(NT):
                    nc.scalar.activation(hd[:, m, n * 512:(n + 1) * 512], pms[n],
                                         SILU, bias=bs[:, m:m + 1])

        def tail(c, it=0):
            # L4: hC -> oT, then transpose to batch-major and store
            for n in range(NT):
                ns_ = slice(n * 512, (n + 1) * 512)
                p4 = mps.tile([D, 512], F32, tag=f"mm{n}", name=f"p4_{it}_{c}_{n}")
                for k in range(KT):
                    nc.tensor.matmul(p4[:], w4r[:, k, :], hC[:, k, ns_],
                                     start=(k == 0), stop=(k == KT - 1))
                nc.vector.tensor_scalar_add(oT[:, ns_], p4[:], b4s)
            for j in range(KT):
                tq = tpp.tile([128, D], F32, tag="tp", name=f"tq_{it}_{c}_{j}")
                nc.tensor.transpose(tq[:], oT[:, j * 128:(j + 1) * 128], ident[:D, :D])
                nc.vector.tensor_copy(oball[:, c % 2, j, :], tq[:])
            nc.sync.dma_start(yv[c], oball[:, c % 2])

        def full_pass(it=0):
            if iters != 1:
                dma_in(0, it)
            transposes_in(0, it)
            for mp in range(KT // 2):
                l1_pair(0, mp, it)
            for c in range(NCH):
                if c + 1 < NCH:
                    dma_in(c + 1, it)
                for m in range(KT):
                    mid_group(2, w2r, b2s, hA, hB, c, m, it)
                for m in range(KT):
                    mid_group(3, w3r, b3s, hB, hC, c, m, it)
                    if c + 1 < NCH:
                        if m == 0:
                            transposes_in(c + 1, it)
                        if m % 2 == 1:
                            l1_pair(c + 1, m // 2, it)
                tail(c, it)

        if iters == 1:
            full_pass()
        else:
            with tc.For_i(0, iters, 1):
                full_pass()

    nc.compile()
    return nc


def _prep_in_maps(t, u, W1, b1, W2, b2, W3, b3, W4, b4):
    in_maps = []
    for e in range(N_ENV):
        w1p = np.empty((128, H), np.float32)
        w1p[:D] = W1[e]
        w1p[D:] = W1[e]
        in_maps.append({
            "x": np.ascontiguousarray(u[e::N_ENV]),
            "w1": w1p,
            "w2": np.ascontiguousarray(W2[e].reshape(KT, 128, H).transpose(1, 0, 2)),
            "w3": np.ascontiguousarray(W3[e].reshape(KT, 128, H).transpose(1, 0, 2)),
            "w4": np.ascontiguousarray(W4[e].reshape(KT, 128, D).transpose(1, 0, 2)),
            "b1": np.ascontiguousarray(b1[e].reshape(KT, 128).T),
            "b2": np.ascontiguousarray(b2[e].reshape(KT, 128).T),
            "b3": np.ascontiguousarray(b3[e].reshape(KT, 128).T),
            "b4": np.ascontiguousarray(b4[e].reshape(D, 1)),
        })
    return in_maps


_CACHED_NC = None


def kernel(t, u, W1, b1, W2, b2, W3, b3, W4, b4):
    global _CACHED_NC
    u = np.asarray(u, np.float32)
    args = [np.asarray(a, np.float32) for a in (W1, b1, W2, b2, W3, b3, W4, b4)]
    if _CACHED_NC is None:
        _CACHED_NC = build_module()
    in_maps = _prep_in_maps(None, u, *args)
    res = run_bass_kernel_spmd(_CACHED_NC, in_maps, core_ids=list(range(N_ENV)))
    out = np.empty((B * N_ENV, D), np.float32)
    for e in range(N_ENV):
        out[e::N_ENV] = res.results[e]["y"]
    return out
